# revision 21
# baseline (speedup 1.0000x reference)
"""Trainium2 Bass kernel for nn_CentroidLayer (vq_codebook).

reference semantics:
    q = l2norm(query_emb); c = l2norm(centroid_emb)
    logits = q @ c.T                       (B, P) cosine sims
    hard = argmax(logits, -1)              (B,) int32
    soft = softmax(logits)                 cancels in forward:
    routing = one_hot(hard) + soft - soft  == one_hot(hard) (+- 1ulp at hot pos)
    context = routing @ centroid_emb       == centroid_emb[hard] (+- 1ulp)

Forward outputs therefore only need the exact argmax:
  - q-normalization is a positive per-row scale -> argmax invariant -> skipped
  - softmax cancels exactly (h+s-s: 0 positions exact, hot position +-2^-24)
  - context is a row gather of the *unnormalized* centroid table

Sharding: data-parallel over B across 8 cores (8192 rows each);
centroid table (1024x512) replicated.

Matmul precision: fp32 matmul runs at 4 cyc/row; fp32r at 1 cyc/row but
with ~2^-13 mantissa. To keep the argmax faithful to the fp32 reference
at fp32r speed we use a compensated 3-pass split:
    x = x_r + x_s,  x_r = f32r(x), x_s = f32r(x - x_r)
    q.c ~= q_r.c_r + q_r.c_s + q_s.c_r     (dropped term ~2^-27)
24 N=512 matmuls/tile at ~220ns vs 8 fp32 matmuls at ~860ns.
The hi/lo split is taken AFTER the PE transpose (4 fp32 transposes per
tile instead of 8 f32r ones); the lo part is computed by DVE directly
from the transpose PSUM.
"""
import os

# The Bass kernel executes through jax/PJRT on the axon-tunneled trn2
# cores; if a caller pinned JAX_PLATFORMS (e.g. to "cpu" for the
# reference), re-include axon so jax can still see the NeuronCores.
_jp = os.environ.get("JAX_PLATFORMS")
if _jp and "axon" not in _jp:
    os.environ["JAX_PLATFORMS"] = _jp + ",axon"

import numpy as np

import concourse.bass as bass
import concourse.bacc as bacc
import concourse.mybir as mybir
import concourse.tile as tile
from concourse import bass_utils

P = 1024
D = 512
B = 65536
N_CORES = 8
B_LOC = B // N_CORES          # 8192
N_TILES = B_LOC // 128        # 64
N_CTILES = P // 128           # 8
N_DCHUNK = D // 128           # 4

_cache = {}


def _build():
    nc = bacc.Bacc("TRN2", target_bir_lowering=False, debug=False)
    f32 = mybir.dt.float32
    f32r = mybir.dt.float32r
    i32 = mybir.dt.int32
    u32 = mybir.dt.uint32
    AF = mybir.ActivationFunctionType
    OP = mybir.AluOpType

    q_d = nc.dram_tensor("q", [B_LOC, D], f32, kind="ExternalInput").ap()
    c_d = nc.dram_tensor("c", [P, D], f32, kind="ExternalInput").ap()
    ident_d = nc.dram_tensor("ident", [128, 128], f32, kind="ExternalInput").ap()
    iota_d = nc.dram_tensor("iota", [128, P], f32, kind="ExternalInput").ap()

    ctx_d = nc.dram_tensor("ctx", [B_LOC, D], f32, kind="ExternalOutput").ap()
    hard_d = nc.dram_tensor("hard", [B_LOC], i32, kind="ExternalOutput").ap()
    rout_d = nc.dram_tensor("rout", [B_LOC, P], f32, kind="ExternalOutput").ap()

    with tile.TileContext(nc) as tc:
        with tc.tile_pool(name="const", bufs=1) as cpool, \
             tc.tile_pool(name="cn", bufs=6) as cnpool, \
             tc.tile_pool(name="sb", bufs=4) as sb, \
             tc.tile_pool(name="deep", bufs=6) as deep, \
             tc.tile_pool(name="ps", bufs=2, space="PSUM") as ps, \
             tc.tile_pool(name="pst", bufs=2, space="PSUM") as pst:

            # ---------------- PE warm-up: dep-free junk matmuls so the HAM
            # clock-gate reaches 8/8 before the real stream starts
            junk_f = cpool.tile([128, 512], f32, tag="junk_f")
            nc.vector.memset(junk_f[:, :], 0.0)
            junk = cpool.tile([128, 512], f32r, tag="junk")
            nc.vector.tensor_copy(out=junk[:, :], in_=junk_f[:, :])
            for w in range(9):
                jp = pst.tile([128, 512], f32r, tag="qtpr")
                nc.tensor.matmul(out=jp[:, :].bitcast(f32), lhsT=junk[:, 0:128],
                                 rhs=junk[:, :], start=True, stop=True)

            ident_s = cpool.tile([128, 128], f32, tag="ident")
            nc.sync.dma_start(out=ident_s[:, :], in_=ident_d[:, :])
            ident_r = cpool.tile([128, 128], f32r, tag="ident_r")
            nc.scalar.copy(out=ident_r[:, :], in_=ident_s[:, :])
            iota_s = cpool.tile([128, P], f32, tag="iota")
            nc.sync.dma_start(out=iota_s[:, :], in_=iota_d[:, :])

            # ---------------- phase 0: normalize centroids, transpose,
            # split to f32r hi/lo in [d_chunk, P] layout.
            # The sqrt/Newton/reciprocal scalar chain runs ONCE on the
            # batched [128, 8] norms instead of per c-tile, removing ~45
            # dependent small-op latencies from the phase-0 critical path.
            cTr = cpool.tile([128, N_DCHUNK, P], f32r, tag="cTr")
            cTs = cpool.tile([128, N_DCHUNK, P], f32r, tag="cTs")
            ssq_all = cpool.tile([128, N_CTILES], f32, tag="ssq_all")
            c_tiles = []
            for t in range(N_CTILES):
                c_s = cpool.tile([128, D], f32, tag=f"c_in{t}")
                c_tiles.append(c_s)
                nc.sync.dma_start(out=c_s[:, :], in_=c_d[t * 128:(t + 1) * 128, :])
                # chunked sum of squares (close to numpy pairwise order)
                ssq4 = cnpool.tile([128, 4], f32, tag="ssq4")
                sqtmp = cnpool.tile([128, 128], f32, tag="sqtmp")
                for j in range(4):
                    nc.scalar.activation(
                        out=sqtmp[:, :],
                        in_=c_s[:, j * 128:(j + 1) * 128],
                        func=AF.Square, accum_out=ssq4[:, j:j + 1])
                s01 = cnpool.tile([128, 1], f32, tag="s01")
                s23 = cnpool.tile([128, 1], f32, tag="s23")
                nc.vector.tensor_tensor(out=s01[:, :], in0=ssq4[:, 0:1], in1=ssq4[:, 1:2], op=OP.add)
                nc.vector.tensor_tensor(out=s23[:, :], in0=ssq4[:, 2:3], in1=ssq4[:, 3:4], op=OP.add)
                nc.vector.tensor_tensor(out=ssq_all[:, t:t + 1], in0=s01[:, :],
                                        in1=s23[:, :], op=OP.add)
            # norm = sqrt(ssq) + one Newton step (ACT LUT is only ~7e-6),
            # batched over all 8 c-tiles
            n0 = cpool.tile([128, N_CTILES], f32, tag="n0")
            nc.scalar.activation(out=n0[:, :], in_=ssq_all[:, :], func=AF.Sqrt)
            r0 = cpool.tile([128, N_CTILES], f32, tag="r0")
            nc.vector.reciprocal(out=r0[:, :], in_=n0[:, :])
            quo = cpool.tile([128, N_CTILES], f32, tag="quo")
            nc.vector.tensor_tensor(out=quo[:, :], in0=ssq_all[:, :], in1=r0[:, :], op=OP.mult)
            nsum = cpool.tile([128, N_CTILES], f32, tag="nsum")
            nc.vector.tensor_tensor(out=nsum[:, :], in0=n0[:, :], in1=quo[:, :], op=OP.add)
            norm = cpool.tile([128, N_CTILES], f32, tag="norm")
            nc.vector.tensor_scalar(out=norm[:, :], in0=nsum[:, :], scalar1=0.5,
                                    scalar2=None, op0=OP.mult)
            rinv = cpool.tile([128, N_CTILES], f32, tag="rinv")
            nc.vector.reciprocal(out=rinv[:, :], in_=norm[:, :])
            for t in range(N_CTILES):
                ts0, ts1 = t * 128, (t + 1) * 128
                c_s = c_tiles[t]
                cn_s = cnpool.tile([128, D], f32, tag="cn_s")
                nc.vector.tensor_scalar(out=cn_s[:, :], in0=c_s[:, :],
                                        scalar1=rinv[:, t:t + 1],
                                        scalar2=None, op0=OP.mult)
                # f32r hi/lo split, then transpose both splits
                chi = cnpool.tile([128, D], f32r, tag="chi")
                nc.scalar.copy(out=chi[:, :], in_=cn_s[:, :])
                clo = cnpool.tile([128, D], f32r, tag="clo")
                nc.vector.tensor_tensor(out=clo[:, :], in0=cn_s[:, :], in1=chi[:, :],
                                        op=OP.subtract)
                ctpr = pst.tile([128, 4 * 128], f32r, tag="qtpr")
                ctps = pst.tile([128, 4 * 128], f32r, tag="qtps")
                for j in range(N_DCHUNK):
                    nc.tensor.transpose(out=ctpr[:, j * 128:(j + 1) * 128],
                                        in_=chi[:, j * 128:(j + 1) * 128],
                                        identity=ident_r[:, :])
                    nc.tensor.transpose(out=ctps[:, j * 128:(j + 1) * 128],
                                        in_=clo[:, j * 128:(j + 1) * 128],
                                        identity=ident_r[:, :])
                nc.scalar.copy(out=cTr[:, :, ts0:ts1],
                               in_=ctpr[:, :].rearrange("p (j b) -> p j b", j=4))
                nc.scalar.copy(out=cTs[:, :, ts0:ts1],
                               in_=ctps[:, :].rearrange("p (j b) -> p j b", j=4))

            # ---------------- phase 1: main loop over 64 q tiles
            hard_acc = cpool.tile([128, N_TILES], f32, tag="hard_acc")
            for t in range(N_TILES):
                r0_, r1_ = t * 128, (t + 1) * 128
                q_s = deep.tile([128, D], f32, tag="q")
                nc.sync.dma_start(out=q_s[:, :], in_=q_d[r0_:r1_, :])

                # f32r hi/lo split of q, then transpose both splits
                qhi = deep.tile([128, D], f32r, tag="qhi")
                nc.scalar.copy(out=qhi[:, :], in_=q_s[:, :])
                qlo = deep.tile([128, D], f32r, tag="qlo")
                nc.vector.tensor_tensor(out=qlo[:, :], in0=q_s[:, :], in1=qhi[:, :],
                                        op=OP.subtract)
                qtpr = pst.tile([128, 4 * 128], f32r, tag="qtpr")
                qtps = pst.tile([128, 4 * 128], f32r, tag="qtps")
                for j in range(N_DCHUNK):
                    nc.tensor.transpose(out=qtpr[:, j * 128:(j + 1) * 128],
                                        in_=qhi[:, j * 128:(j + 1) * 128],
                                        identity=ident_r[:, :])
                    nc.tensor.transpose(out=qtps[:, j * 128:(j + 1) * 128],
                                        in_=qlo[:, j * 128:(j + 1) * 128],
                                        identity=ident_r[:, :])
                qTr = deep.tile([128, 4, 128], f32r, tag="qTr")
                nc.scalar.copy(out=qTr[:, :, :], in_=qtpr[:, :].rearrange("p (j b) -> p j b", j=4))
                qTs = deep.tile([128, 4, 128], f32r, tag="qTs")
                nc.scalar.copy(out=qTs[:, :, :], in_=qtps[:, :].rearrange("p (j b) -> p j b", j=4))

                # logits tile [128, 1024] in PSUM: per half, a 12-matmul
                # accumulation chain (qhi.chi + qhi.clo + qlo.chi)
                lg = ps.tile([128, P], f32, tag="lg")
                for h in range(2):
                    hs = slice(h * 512, (h + 1) * 512)
                    k = 0
                    for (lhsT, rhs) in ((qTr, cTr), (qTr, cTs), (qTs, cTr)):
                        for j in range(N_DCHUNK):
                            nc.tensor.matmul(
                                out=lg[:, hs],
                                lhsT=lhsT[:, j, :],
                                rhs=rhs[:, j, hs],
                                start=(k == 0), stop=(k == 11))
                            k += 1
                lg_s = sb.tile([128, P], f32, tag="lg_s")
                nc.scalar.copy(out=lg_s[:, :], in_=lg[:, :])

                # argmax
                max8 = sb.tile([128, 8], f32, tag="max8")
                idx8 = sb.tile([128, 8], u32, tag="idx8")
                nc.vector.max(out=max8[:, :], in_=lg_s[:, :])
                nc.vector.max_index(out=idx8[:, :], in_max=max8[:, :], in_values=lg_s[:, :])
                idxu = sb.tile([128, 1], u32, tag="idxu")
                nc.vector.tensor_copy(out=idxu[:, :], in_=idx8[:, 0:1])
                idxf = sb.tile([128, 1], f32, tag="idxf")
                nc.vector.tensor_copy(out=idxf[:, :], in_=idxu[:, :])
                nc.vector.tensor_copy(out=hard_acc[:, t:t + 1], in_=idxf[:, :])

                # routing one-hot
                rt = sb.tile([128, P], f32, tag="rt")
                nc.vector.tensor_scalar(out=rt[:, :], in0=iota_s[:, :], scalar1=idxf[:, :],
                                        scalar2=None, op0=OP.is_equal)
                nc.scalar.dma_start(out=rout_d[r0_:r1_, :], in_=rt[:, :])

                # context gather
                ctx_s = sb.tile([128, D], f32, tag="ctx")
                nc.gpsimd.indirect_dma_start(
                    out=ctx_s[:, :], out_offset=None,
                    in_=c_d[:, :],
                    in_offset=bass.IndirectOffsetOnAxis(ap=idxu[:, :], axis=0))
                nc.gpsimd.dma_start(out=ctx_d[r0_:r1_, :], in_=ctx_s[:, :])

            # ---------------- phase 2: hard assignment output
            htp = pst.tile([128, 128], f32r, tag="qtpr")
            nc.tensor.transpose(out=htp[0:64, 0:128].bitcast(f32), in_=hard_acc[:, :],
                                identity=ident_s[:, :])
            hard_i = cpool.tile([64, 128], i32, tag="hard_i")
            nc.vector.tensor_copy(out=hard_i[:, :], in_=htp[0:64, 0:128].bitcast(f32))
            nc.sync.dma_start(
                out=hard_d.rearrange("(t p) -> t p", p=128),
                in_=hard_i[:, :])

    nc.compile()
    return nc


def _get_nc():
    if "nc" not in _cache:
        _cache["nc"] = _build()
    return _cache["nc"]


def kernel(query_emb: np.ndarray, centroid_emb: np.ndarray, *, _trace=False, _trace_kwargs=None):
    nc = _get_nc()
    q = np.ascontiguousarray(query_emb, dtype=np.float32)
    c = np.ascontiguousarray(centroid_emb, dtype=np.float32)
    ident = np.eye(128, dtype=np.float32)
    iota = np.broadcast_to(np.arange(P, dtype=np.float32), (128, P)).copy()

    in_maps = []
    for k in range(N_CORES):
        in_maps.append({
            "q": q[k * B_LOC:(k + 1) * B_LOC],
            "c": c,
            "ident": ident,
            "iota": iota,
        })
    res = bass_utils.run_bass_kernel_spmd(
        nc, in_maps, core_ids=list(range(N_CORES)),
        trace=_trace, **(_trace_kwargs or {}))

    context = np.concatenate([res.results[k]["ctx"] for k in range(N_CORES)], axis=0)
    hard = np.concatenate([res.results[k]["hard"] for k in range(N_CORES)], axis=0)
    routing = np.concatenate([res.results[k]["rout"] for k in range(N_CORES)], axis=0)
    if _trace:
        return (context, hard, routing), res
    return context, hard, routing


# revision 23
# speedup vs baseline: 1.0687x; 1.0687x over previous
"""Trainium2 Bass kernel for nn_CentroidLayer (vq_codebook).

reference semantics:
    q = l2norm(query_emb); c = l2norm(centroid_emb)
    logits = q @ c.T                       (B, P) cosine sims
    hard = argmax(logits, -1)              (B,) int32
    soft = softmax(logits)                 cancels in forward:
    routing = one_hot(hard) + soft - soft  == one_hot(hard) (+- 1ulp at hot pos)
    context = routing @ centroid_emb       == centroid_emb[hard] (+- 1ulp)

Forward outputs therefore only need the exact argmax:
  - q-normalization is a positive per-row scale -> argmax invariant -> skipped
  - softmax cancels exactly (h+s-s: 0 positions exact, hot position +-2^-24)
  - context is a row gather of the *unnormalized* centroid table

Sharding: data-parallel over B across 8 cores (8192 rows each);
centroid table (1024x512) replicated.

Matmul precision: fp32 matmul runs at 4 cyc/row; fp32r at 1 cyc/row but
with ~2^-13 mantissa. To keep the argmax faithful to the fp32 reference
at fp32r speed we use a compensated 3-pass split:
    x = x_r + x_s,  x_r = f32r(x), x_s = f32r(x - x_r)
    q.c ~= q_r.c_r + q_r.c_s + q_s.c_r     (dropped term ~2^-27)
24 N=512 matmuls/tile at ~220ns vs 8 fp32 matmuls at ~860ns.
The tile is transposed ONCE in fp32 (4 PE transposes); the f32r hi part
is an ACT cast-copy from the transpose PSUM and the lo part a DVE
subtract against it. The hi-part matmul passes run first so the lo part
has ~8 matmuls of slack before it is consumed.
"""
import os

# The Bass kernel executes through jax/PJRT on the axon-tunneled trn2
# cores; if a caller pinned JAX_PLATFORMS (e.g. to "cpu" for the
# reference), re-include axon so jax can still see the NeuronCores.
_jp = os.environ.get("JAX_PLATFORMS")
if _jp and "axon" not in _jp:
    os.environ["JAX_PLATFORMS"] = _jp + ",axon"

import numpy as np

import concourse.bass as bass
import concourse.bacc as bacc
import concourse.mybir as mybir
import concourse.tile as tile
from concourse import bass_utils

P = 1024
D = 512
B = 65536
N_CORES = 8
B_LOC = B // N_CORES          # 8192
N_TILES = B_LOC // 128        # 64
N_CTILES = P // 128           # 8
N_DCHUNK = D // 128           # 4

_cache = {}


def _build():
    nc = bacc.Bacc("TRN2", target_bir_lowering=False, debug=False)
    f32 = mybir.dt.float32
    f32r = mybir.dt.float32r
    i32 = mybir.dt.int32
    u32 = mybir.dt.uint32
    AF = mybir.ActivationFunctionType
    OP = mybir.AluOpType

    q_d = nc.dram_tensor("q", [B_LOC, D], f32, kind="ExternalInput").ap()
    c_d = nc.dram_tensor("c", [P, D], f32, kind="ExternalInput").ap()
    ident_d = nc.dram_tensor("ident", [128, 128], f32, kind="ExternalInput").ap()
    iota_d = nc.dram_tensor("iota", [128, P], f32, kind="ExternalInput").ap()

    ctx_d = nc.dram_tensor("ctx", [B_LOC, D], f32, kind="ExternalOutput").ap()
    hard_d = nc.dram_tensor("hard", [B_LOC], i32, kind="ExternalOutput").ap()
    rout_d = nc.dram_tensor("rout", [B_LOC, P], f32, kind="ExternalOutput").ap()

    with tile.TileContext(nc) as tc:
        with tc.tile_pool(name="const", bufs=1) as cpool, \
             tc.tile_pool(name="cn", bufs=6) as cnpool, \
             tc.tile_pool(name="sb", bufs=4) as sb, \
             tc.tile_pool(name="deep", bufs=6) as deep, \
             tc.tile_pool(name="ps", bufs=2, space="PSUM") as ps, \
             tc.tile_pool(name="pst", bufs=3, space="PSUM") as pst:

            # ---------------- PE warm-up: dep-free junk matmuls so the HAM
            # clock-gate reaches 8/8 before the real stream starts
            junk = cpool.tile([128, 512], f32, tag="junk")
            nc.vector.memset(junk[:, :], 0.0)
            for w in range(9):
                jp = pst.tile([128, 512], f32, tag="qtp")
                nc.tensor.matmul(out=jp[:, :], lhsT=junk[:, 0:128],
                                 rhs=junk[:, :], start=True, stop=True)

            ident_s = cpool.tile([128, 128], f32, tag="ident")
            nc.sync.dma_start(out=ident_s[:, :], in_=ident_d[:, :])
            iota_s = cpool.tile([128, P], f32, tag="iota")
            nc.sync.dma_start(out=iota_s[:, :], in_=iota_d[:, :])

            # ---------------- phase 0: normalize centroids, transpose,
            # split to f32r hi/lo in [d_chunk, P] layout
            cTr = cpool.tile([128, N_DCHUNK, P], f32r, tag="cTr")
            cTs = cpool.tile([128, N_DCHUNK, P], f32r, tag="cTs")
            for t in range(N_CTILES):
                ts0, ts1 = t * 128, (t + 1) * 128
                c_s = cnpool.tile([128, D], f32, tag="c_in")
                nc.sync.dma_start(out=c_s[:, :], in_=c_d[ts0:ts1, :])
                # chunked sum of squares (close to numpy pairwise order)
                ssq4 = cnpool.tile([128, 4], f32, tag="ssq4")
                sqtmp = cnpool.tile([128, 128], f32, tag="sqtmp")
                for j in range(4):
                    nc.scalar.activation(
                        out=sqtmp[:, :],
                        in_=c_s[:, j * 128:(j + 1) * 128],
                        func=AF.Square, accum_out=ssq4[:, j:j + 1])
                s01 = cnpool.tile([128, 1], f32, tag="s01")
                s23 = cnpool.tile([128, 1], f32, tag="s23")
                ssq = cnpool.tile([128, 1], f32, tag="ssq")
                nc.vector.tensor_tensor(out=s01[:, :], in0=ssq4[:, 0:1], in1=ssq4[:, 1:2], op=OP.add)
                nc.vector.tensor_tensor(out=s23[:, :], in0=ssq4[:, 2:3], in1=ssq4[:, 3:4], op=OP.add)
                nc.vector.tensor_tensor(out=ssq[:, :], in0=s01[:, :], in1=s23[:, :], op=OP.add)
                # norm = sqrt(ssq) + one Newton step (ACT LUT is only ~7e-6)
                n0 = cnpool.tile([128, 1], f32, tag="n0")
                nc.scalar.activation(out=n0[:, :], in_=ssq[:, :], func=AF.Sqrt)
                r0 = cnpool.tile([128, 1], f32, tag="r0")
                nc.vector.reciprocal(out=r0[:, :], in_=n0[:, :])
                quo = cnpool.tile([128, 1], f32, tag="quo")
                nc.vector.tensor_tensor(out=quo[:, :], in0=ssq[:, :], in1=r0[:, :], op=OP.mult)
                nsum = cnpool.tile([128, 1], f32, tag="nsum")
                nc.vector.tensor_tensor(out=nsum[:, :], in0=n0[:, :], in1=quo[:, :], op=OP.add)
                norm = cnpool.tile([128, 1], f32, tag="norm")
                nc.vector.tensor_scalar(out=norm[:, :], in0=nsum[:, :], scalar1=0.5,
                                        scalar2=None, op0=OP.mult)
                rinv = cnpool.tile([128, 1], f32, tag="rinv")
                nc.vector.reciprocal(out=rinv[:, :], in_=norm[:, :])
                cn_s = cnpool.tile([128, D], f32, tag="cn_s")
                nc.vector.tensor_scalar(out=cn_s[:, :], in0=c_s[:, :], scalar1=rinv[:, :],
                                        scalar2=None, op0=OP.mult)
                # transpose fp32, then hi = ACT f32r cast, lo = DVE subtract
                ctp = pst.tile([128, 4 * 128], f32, tag="qtp")
                for j in range(N_DCHUNK):
                    nc.tensor.transpose(out=ctp[:, j * 128:(j + 1) * 128],
                                        in_=cn_s[:, j * 128:(j + 1) * 128],
                                        identity=ident_s[:, :])
                ctp3 = ctp[:, :].rearrange("p (j b) -> p j b", j=4)
                nc.scalar.copy(out=cTr[:, :, ts0:ts1], in_=ctp3)
                nc.vector.tensor_tensor(out=cTs[:, :, ts0:ts1], in0=ctp3,
                                        in1=cTr[:, :, ts0:ts1], op=OP.subtract)

            # ---------------- phase 1: main loop over 64 q tiles
            hard_acc = cpool.tile([128, N_TILES], f32, tag="hard_acc")
            for t in range(N_TILES):
                r0_, r1_ = t * 128, (t + 1) * 128
                q_s = deep.tile([128, D], f32, tag="q")
                nc.sync.dma_start(out=q_s[:, :], in_=q_d[r0_:r1_, :])

                # transpose fp32 q tile once -> [d_chunk partitions, batch],
                # then split: hi = ACT cast from PSUM, lo = DVE subtract
                qtp = pst.tile([128, 4 * 128], f32, tag="qtp")
                for j in range(N_DCHUNK):
                    nc.tensor.transpose(out=qtp[:, j * 128:(j + 1) * 128],
                                        in_=q_s[:, j * 128:(j + 1) * 128],
                                        identity=ident_s[:, :])
                qtp3 = qtp[:, :].rearrange("p (j b) -> p j b", j=4)
                qTr = deep.tile([128, 4, 128], f32r, tag="qTr")
                nc.scalar.copy(out=qTr[:, :, :], in_=qtp3)
                qTs = deep.tile([128, 4, 128], f32r, tag="qTs")
                nc.vector.tensor_tensor(out=qTs[:, :, :], in0=qtp3, in1=qTr[:, :, :],
                                        op=OP.subtract)

                # logits tile [128, 1024] in PSUM: per half, a 12-matmul
                # accumulation chain; hi-part passes first so the DVE
                # subtract producing qTs has slack
                lg = ps.tile([128, P], f32, tag="lg")
                for h in range(2):
                    hs = slice(h * 512, (h + 1) * 512)
                    k = 0
                    for (lhsT, rhs) in ((qTr, cTr), (qTr, cTs), (qTs, cTr)):
                        for j in range(N_DCHUNK):
                            nc.tensor.matmul(
                                out=lg[:, hs],
                                lhsT=lhsT[:, j, :],
                                rhs=rhs[:, j, hs],
                                start=(k == 0), stop=(k == 11))
                            k += 1
                lg_s = sb.tile([128, P], f32, tag="lg_s")
                nc.scalar.copy(out=lg_s[:, :], in_=lg[:, :])

                # argmax
                max8 = sb.tile([128, 8], f32, tag="max8")
                idx8 = sb.tile([128, 8], u32, tag="idx8")
                nc.vector.max(out=max8[:, :], in_=lg_s[:, :])
                nc.vector.max_index(out=idx8[:, :], in_max=max8[:, :], in_values=lg_s[:, :])
                idxu = sb.tile([128, 1], u32, tag="idxu")
                nc.vector.tensor_copy(out=idxu[:, :], in_=idx8[:, 0:1])
                idxf = sb.tile([128, 1], f32, tag="idxf")
                nc.vector.tensor_copy(out=idxf[:, :], in_=idxu[:, :])
                nc.vector.tensor_copy(out=hard_acc[:, t:t + 1], in_=idxf[:, :])

                # routing one-hot
                rt = sb.tile([128, P], f32, tag="rt")
                nc.vector.tensor_scalar(out=rt[:, :], in0=iota_s[:, :], scalar1=idxf[:, :],
                                        scalar2=None, op0=OP.is_equal)
                nc.scalar.dma_start(out=rout_d[r0_:r1_, :], in_=rt[:, :])

                # context gather (SWDGE queue, decoupled from sync-ring q loads)
                ctx_s = sb.tile([128, D], f32, tag="ctx")
                nc.gpsimd.indirect_dma_start(
                    out=ctx_s[:, :], out_offset=None,
                    in_=c_d[:, :],
                    in_offset=bass.IndirectOffsetOnAxis(ap=idxu[:, :], axis=0))
                nc.gpsimd.dma_start(out=ctx_d[r0_:r1_, :], in_=ctx_s[:, :])

            # ---------------- phase 2: hard assignment output
            htp = pst.tile([128, 128], f32, tag="qtp")
            nc.tensor.transpose(out=htp[0:64, 0:128], in_=hard_acc[:, :],
                                identity=ident_s[:, :])
            hard_i = cpool.tile([64, 128], i32, tag="hard_i")
            nc.vector.tensor_copy(out=hard_i[:, :], in_=htp[0:64, 0:128])
            nc.sync.dma_start(
                out=hard_d.rearrange("(t p) -> t p", p=128),
                in_=hard_i[:, :])

    nc.compile()
    return nc


def _get_nc():
    if "nc" not in _cache:
        _cache["nc"] = _build()
    return _cache["nc"]


def kernel(query_emb: np.ndarray, centroid_emb: np.ndarray, *, _trace=False, _trace_kwargs=None):
    nc = _get_nc()
    q = np.ascontiguousarray(query_emb, dtype=np.float32)
    c = np.ascontiguousarray(centroid_emb, dtype=np.float32)
    ident = np.eye(128, dtype=np.float32)
    iota = np.broadcast_to(np.arange(P, dtype=np.float32), (128, P)).copy()

    in_maps = []
    for k in range(N_CORES):
        in_maps.append({
            "q": q[k * B_LOC:(k + 1) * B_LOC],
            "c": c,
            "ident": ident,
            "iota": iota,
        })
    res = bass_utils.run_bass_kernel_spmd(
        nc, in_maps, core_ids=list(range(N_CORES)),
        trace=_trace, **(_trace_kwargs or {}))

    context = np.concatenate([res.results[k]["ctx"] for k in range(N_CORES)], axis=0)
    hard = np.concatenate([res.results[k]["hard"] for k in range(N_CORES)], axis=0)
    routing = np.concatenate([res.results[k]["rout"] for k in range(N_CORES)], axis=0)
    if _trace:
        return (context, hard, routing), res
    return context, hard, routing


# revision 24
# speedup vs baseline: 1.0712x; 1.0024x over previous
"""Trainium2 Bass kernel for nn_CentroidLayer (vq_codebook).

reference semantics:
    q = l2norm(query_emb); c = l2norm(centroid_emb)
    logits = q @ c.T                       (B, P) cosine sims
    hard = argmax(logits, -1)              (B,) int32
    soft = softmax(logits)                 cancels in forward:
    routing = one_hot(hard) + soft - soft  == one_hot(hard) (+- 1ulp at hot pos)
    context = routing @ centroid_emb       == centroid_emb[hard] (+- 1ulp)

Forward outputs therefore only need the exact argmax:
  - q-normalization is a positive per-row scale -> argmax invariant -> skipped
  - softmax cancels exactly (h+s-s: 0 positions exact, hot position +-2^-24)
  - context is a row gather of the *unnormalized* centroid table

Sharding: data-parallel over B across 8 cores (8192 rows each);
centroid table (1024x512) replicated.

Matmul precision: fp32 matmul runs at 4 cyc/row; fp32r at 1 cyc/row but
with ~2^-13 mantissa. To keep the argmax faithful to the fp32 reference
at fp32r speed we use a compensated 3-pass split:
    x = x_r + x_s,  x_r = f32r(x), x_s = f32r(x - x_r)
    q.c ~= q_r.c_r + q_r.c_s + q_s.c_r     (dropped term ~2^-27)
24 N=512 matmuls/tile at ~220ns vs 8 fp32 matmuls at ~860ns.
The tile is transposed ONCE in fp32 (4 PE transposes); the f32r hi part
is an ACT cast-copy from the transpose PSUM and the lo part a DVE
subtract against it. The hi-part matmul passes run first so the lo part
has ~8 matmuls of slack before it is consumed.
"""
import os

# The Bass kernel executes through jax/PJRT on the axon-tunneled trn2
# cores; if a caller pinned JAX_PLATFORMS (e.g. to "cpu" for the
# reference), re-include axon so jax can still see the NeuronCores.
_jp = os.environ.get("JAX_PLATFORMS")
if _jp and "axon" not in _jp:
    os.environ["JAX_PLATFORMS"] = _jp + ",axon"

import numpy as np

import concourse.bass as bass
import concourse.bacc as bacc
import concourse.mybir as mybir
import concourse.tile as tile
from concourse import bass_utils

P = 1024
D = 512
B = 65536
N_CORES = 8
B_LOC = B // N_CORES          # 8192
N_TILES = B_LOC // 128        # 64
N_CTILES = P // 128           # 8
N_DCHUNK = D // 128           # 4

_cache = {}


def _build():
    nc = bacc.Bacc("TRN2", target_bir_lowering=False, debug=False)
    f32 = mybir.dt.float32
    f32r = mybir.dt.float32r
    i32 = mybir.dt.int32
    u32 = mybir.dt.uint32
    AF = mybir.ActivationFunctionType
    OP = mybir.AluOpType

    q_d = nc.dram_tensor("q", [B_LOC, D], f32, kind="ExternalInput").ap()
    c_d = nc.dram_tensor("c", [P, D], f32, kind="ExternalInput").ap()
    ident_d = nc.dram_tensor("ident", [128, 128], f32, kind="ExternalInput").ap()
    iota_d = nc.dram_tensor("iota", [128, P], f32, kind="ExternalInput").ap()

    ctx_d = nc.dram_tensor("ctx", [B_LOC, D], f32, kind="ExternalOutput").ap()
    hard_d = nc.dram_tensor("hard", [B_LOC], i32, kind="ExternalOutput").ap()
    rout_d = nc.dram_tensor("rout", [B_LOC, P], f32, kind="ExternalOutput").ap()

    with tile.TileContext(nc) as tc:
        with tc.tile_pool(name="const", bufs=1) as cpool, \
             tc.tile_pool(name="cn", bufs=6) as cnpool, \
             tc.tile_pool(name="sb", bufs=4) as sb, \
             tc.tile_pool(name="deep", bufs=9) as deep, \
             tc.tile_pool(name="ps", bufs=2, space="PSUM") as ps, \
             tc.tile_pool(name="pst", bufs=4, space="PSUM") as pst:

            # ---------------- PE warm-up: dep-free junk matmuls so the HAM
            # clock-gate reaches 8/8 before the real stream starts
            junk = cpool.tile([128, 512], f32, tag="junk")
            nc.vector.memset(junk[:, :], 0.0)
            for w in range(9):
                jp = pst.tile([128, 512], f32, tag="qtp")
                nc.tensor.matmul(out=jp[:, :], lhsT=junk[:, 0:128],
                                 rhs=junk[:, :], start=True, stop=True)

            ident_s = cpool.tile([128, 128], f32, tag="ident")
            nc.sync.dma_start(out=ident_s[:, :], in_=ident_d[:, :])
            iota_s = cpool.tile([128, P], f32, tag="iota")
            nc.sync.dma_start(out=iota_s[:, :], in_=iota_d[:, :])

            # ---------------- phase 0: normalize centroids, transpose,
            # split to f32r hi/lo in [d_chunk, P] layout
            cTr = cpool.tile([128, N_DCHUNK, P], f32r, tag="cTr")
            cTs = cpool.tile([128, N_DCHUNK, P], f32r, tag="cTs")
            for t in range(N_CTILES):
                ts0, ts1 = t * 128, (t + 1) * 128
                c_s = cnpool.tile([128, D], f32, tag="c_in")
                nc.sync.dma_start(out=c_s[:, :], in_=c_d[ts0:ts1, :])
                # chunked sum of squares (close to numpy pairwise order)
                ssq4 = cnpool.tile([128, 4], f32, tag="ssq4")
                sqtmp = cnpool.tile([128, 128], f32, tag="sqtmp")
                for j in range(4):
                    nc.scalar.activation(
                        out=sqtmp[:, :],
                        in_=c_s[:, j * 128:(j + 1) * 128],
                        func=AF.Square, accum_out=ssq4[:, j:j + 1])
                s01 = cnpool.tile([128, 1], f32, tag="s01")
                s23 = cnpool.tile([128, 1], f32, tag="s23")
                ssq = cnpool.tile([128, 1], f32, tag="ssq")
                nc.vector.tensor_tensor(out=s01[:, :], in0=ssq4[:, 0:1], in1=ssq4[:, 1:2], op=OP.add)
                nc.vector.tensor_tensor(out=s23[:, :], in0=ssq4[:, 2:3], in1=ssq4[:, 3:4], op=OP.add)
                nc.vector.tensor_tensor(out=ssq[:, :], in0=s01[:, :], in1=s23[:, :], op=OP.add)
                # norm = sqrt(ssq) + one Newton step (ACT LUT is only ~7e-6)
                n0 = cnpool.tile([128, 1], f32, tag="n0")
                nc.scalar.activation(out=n0[:, :], in_=ssq[:, :], func=AF.Sqrt)
                r0 = cnpool.tile([128, 1], f32, tag="r0")
                nc.vector.reciprocal(out=r0[:, :], in_=n0[:, :])
                quo = cnpool.tile([128, 1], f32, tag="quo")
                nc.vector.tensor_tensor(out=quo[:, :], in0=ssq[:, :], in1=r0[:, :], op=OP.mult)
                nsum = cnpool.tile([128, 1], f32, tag="nsum")
                nc.vector.tensor_tensor(out=nsum[:, :], in0=n0[:, :], in1=quo[:, :], op=OP.add)
                norm = cnpool.tile([128, 1], f32, tag="norm")
                nc.vector.tensor_scalar(out=norm[:, :], in0=nsum[:, :], scalar1=0.5,
                                        scalar2=None, op0=OP.mult)
                rinv = cnpool.tile([128, 1], f32, tag="rinv")
                nc.vector.reciprocal(out=rinv[:, :], in_=norm[:, :])
                cn_s = cnpool.tile([128, D], f32, tag="cn_s")
                nc.vector.tensor_scalar(out=cn_s[:, :], in0=c_s[:, :], scalar1=rinv[:, :],
                                        scalar2=None, op0=OP.mult)
                # transpose fp32, then hi = ACT f32r cast, lo = DVE subtract
                ctp = pst.tile([128, 4 * 128], f32, tag="qtp")
                for j in range(N_DCHUNK):
                    nc.tensor.transpose(out=ctp[:, j * 128:(j + 1) * 128],
                                        in_=cn_s[:, j * 128:(j + 1) * 128],
                                        identity=ident_s[:, :])
                ctp3 = ctp[:, :].rearrange("p (j b) -> p j b", j=4)
                nc.scalar.copy(out=cTr[:, :, ts0:ts1], in_=ctp3)
                nc.vector.tensor_tensor(out=cTs[:, :, ts0:ts1], in0=ctp3,
                                        in1=cTr[:, :, ts0:ts1], op=OP.subtract)

            # ---------------- phase 1: main loop over 64 q tiles
            hard_acc = cpool.tile([128, N_TILES], f32, tag="hard_acc")
            for t in range(N_TILES):
                r0_, r1_ = t * 128, (t + 1) * 128
                q_s = deep.tile([128, D], f32, tag="q")
                nc.sync.dma_start(out=q_s[:, :], in_=q_d[r0_:r1_, :])

                # transpose fp32 q tile once -> [d_chunk partitions, batch],
                # then split: hi = ACT cast from PSUM, lo = DVE subtract
                qtp = pst.tile([128, 4 * 128], f32, tag="qtp")
                for j in range(N_DCHUNK):
                    nc.tensor.transpose(out=qtp[:, j * 128:(j + 1) * 128],
                                        in_=q_s[:, j * 128:(j + 1) * 128],
                                        identity=ident_s[:, :])
                qtp3 = qtp[:, :].rearrange("p (j b) -> p j b", j=4)
                qTr = deep.tile([128, 4, 128], f32r, tag="qTr")
                nc.scalar.copy(out=qTr[:, :, :], in_=qtp3)
                qTs = deep.tile([128, 4, 128], f32r, tag="qTs")
                nc.vector.tensor_tensor(out=qTs[:, :, :], in0=qtp3, in1=qTr[:, :, :],
                                        op=OP.subtract)

                # logits tile [128, 1024] in PSUM: per half, a 12-matmul
                # accumulation chain; hi-part passes first so the DVE
                # subtract producing qTs has slack
                lg = ps.tile([128, P], f32, tag="lg")
                for h in range(2):
                    hs = slice(h * 512, (h + 1) * 512)
                    k = 0
                    for (lhsT, rhs) in ((qTr, cTr), (qTr, cTs), (qTs, cTr)):
                        for j in range(N_DCHUNK):
                            nc.tensor.matmul(
                                out=lg[:, hs],
                                lhsT=lhsT[:, j, :],
                                rhs=rhs[:, j, hs],
                                start=(k == 0), stop=(k == 11))
                            k += 1
                lg_s = sb.tile([128, P], f32, tag="lg_s")
                nc.scalar.copy(out=lg_s[:, :], in_=lg[:, :])

                # argmax
                max8 = sb.tile([128, 8], f32, tag="max8")
                idx8 = sb.tile([128, 8], u32, tag="idx8")
                nc.vector.max(out=max8[:, :], in_=lg_s[:, :])
                nc.vector.max_index(out=idx8[:, :], in_max=max8[:, :], in_values=lg_s[:, :])
                idxu = sb.tile([128, 1], u32, tag="idxu")
                nc.vector.tensor_copy(out=idxu[:, :], in_=idx8[:, 0:1])
                idxf = sb.tile([128, 1], f32, tag="idxf")
                nc.vector.tensor_copy(out=idxf[:, :], in_=idxu[:, :])
                nc.vector.tensor_copy(out=hard_acc[:, t:t + 1], in_=idxf[:, :])

                # routing one-hot
                rt = sb.tile([128, P], f32, tag="rt")
                nc.vector.tensor_scalar(out=rt[:, :], in0=iota_s[:, :], scalar1=idxf[:, :],
                                        scalar2=None, op0=OP.is_equal)
                nc.scalar.dma_start(out=rout_d[r0_:r1_, :], in_=rt[:, :])

                # context gather (SWDGE queue, decoupled from sync-ring q loads)
                ctx_s = sb.tile([128, D], f32, tag="ctx")
                nc.gpsimd.indirect_dma_start(
                    out=ctx_s[:, :], out_offset=None,
                    in_=c_d[:, :],
                    in_offset=bass.IndirectOffsetOnAxis(ap=idxu[:, :], axis=0))
                nc.gpsimd.dma_start(out=ctx_d[r0_:r1_, :], in_=ctx_s[:, :])

            # ---------------- phase 2: hard assignment output
            htp = pst.tile([128, 128], f32, tag="qtp")
            nc.tensor.transpose(out=htp[0:64, 0:128], in_=hard_acc[:, :],
                                identity=ident_s[:, :])
            hard_i = cpool.tile([64, 128], i32, tag="hard_i")
            nc.vector.tensor_copy(out=hard_i[:, :], in_=htp[0:64, 0:128])
            nc.sync.dma_start(
                out=hard_d.rearrange("(t p) -> t p", p=128),
                in_=hard_i[:, :])

    nc.compile()
    return nc


def _get_nc():
    if "nc" not in _cache:
        _cache["nc"] = _build()
    return _cache["nc"]


def kernel(query_emb: np.ndarray, centroid_emb: np.ndarray, *, _trace=False, _trace_kwargs=None):
    nc = _get_nc()
    q = np.ascontiguousarray(query_emb, dtype=np.float32)
    c = np.ascontiguousarray(centroid_emb, dtype=np.float32)
    ident = np.eye(128, dtype=np.float32)
    iota = np.broadcast_to(np.arange(P, dtype=np.float32), (128, P)).copy()

    in_maps = []
    for k in range(N_CORES):
        in_maps.append({
            "q": q[k * B_LOC:(k + 1) * B_LOC],
            "c": c,
            "ident": ident,
            "iota": iota,
        })
    res = bass_utils.run_bass_kernel_spmd(
        nc, in_maps, core_ids=list(range(N_CORES)),
        trace=_trace, **(_trace_kwargs or {}))

    context = np.concatenate([res.results[k]["ctx"] for k in range(N_CORES)], axis=0)
    hard = np.concatenate([res.results[k]["hard"] for k in range(N_CORES)], axis=0)
    routing = np.concatenate([res.results[k]["rout"] for k in range(N_CORES)], axis=0)
    if _trace:
        return (context, hard, routing), res
    return context, hard, routing
